# revision 11
# baseline (speedup 1.0000x reference)
"""Grouped (MoE-style) linear on 8 trn2 NeuronCores.

out[t] = hidden_states[t] @ weight[g(t)], where token t belongs to group g iff
offsets[g-1] <= t < offsets[g] (searchsorted right semantics; tokens at or past
offsets[-1] get zero output).

Strategy: expert-parallel. Core g owns weight[g] and the contiguous token run
of group g. Routing is done host-side (offsets are host data); each core runs
an identical Bass program: [ntb*128, 1024] x [1024, 1024] matmul in bf16
(inputs cast host-side; PSUM accumulation stays fp32, measured ~1.5e-3 relmax
for this distribution -- well inside the 2e-2 gate).

Performance structure (per core, ntb=16):
  - PE floor is 256 N=512 matmuls = 131072 cycles ~= 54.6us at 2.4 GHz; the
    kernel is tensor-bound, so everything else is pipelined around the MM
    stream.
  - k-outer loop over PAIRS of 128-token blocks: each k-step is 4 matmuls
    (2 blocks x 2 out-halves) accumulating into 4 PSUM banks, so compute
    starts as soon as the first 128-row weight chunk lands instead of after
    the full 2MB weight load.
  - X streams on the sync HWDGE ring as one 512KB DMA per block-pair (4KB
    per-partition runs); W streams on the scalar ring as per-k 256KB chunks,
    issued in exactly the order the k-loop consumes them.
  - Outputs drain progressively: per block, ACT copies PSUM[0:512] -> SBUF
    and DMAs it on the sync ring while DVE copies PSUM[512:1024] for the
    scalar ring, so the post-loop tail is one 256KB DMA, not 8MB.
"""
import numpy as np
import ml_dtypes

import concourse.bass as bass
import concourse.tile as tile
from concourse import bacc, mybir
from concourse.bass_utils import run_bass_kernel_spmd

GROUPS = 8
TOKENS = 16384
IN_F = 1024
OUT_F = 1024
KCH = IN_F // 128  # contraction chunks
BF16 = ml_dtypes.bfloat16


def build(ntb: int) -> bass.Bass:
    """One core's program: ntb 128-token blocks through a 1024x1024 expert."""
    f32 = mybir.dt.float32
    bf16 = mybir.dt.bfloat16
    npair = (ntb + 1) // 2
    nc = bacc.Bacc()
    # xt[pair, p, j, k, tok] = X[(2*pair+j)*128 + tok, k*128 + p]
    xt_d = nc.dram_tensor("xt", [npair, 128, 2, KCH, 128], bf16,
                          kind="ExternalInput")
    # x0 duplicates pair 0's k=0 chunk contiguously (512B/partition runs;
    # slicing it out of xt would cost 256B-run descriptors on the hot path)
    x0_d = nc.dram_tensor("x0", [128, 2, 128], bf16, kind="ExternalInput")
    # w[k, p, n] = W[k*128 + p, n]
    w_d = nc.dram_tensor("w", [KCH, 128, OUT_F], bf16, kind="ExternalInput")
    out_d = nc.dram_tensor("out", [ntb * 128, OUT_F], f32,
                           kind="ExternalOutput")

    with tile.TileContext(nc) as tc:
        with (
            tc.tile_pool(name="wp", bufs=1) as wp,
            tc.tile_pool(name="xp", bufs=min(npair, 20)) as xp,
            tc.tile_pool(name="op", bufs=4) as op,
            tc.tile_pool(name="ps", bufs=4, space="PSUM") as psp,
        ):
            wt = wp.tile([128, KCH, OUT_F], bf16)
            # Bootstrap pieces ride the otherwise-idle scalar ring so the
            # first k-step's data (x0 + w[k0]) lands in parallel with the
            # sync ring's stream; the sync ring carries everything else in
            # exactly k-loop consumption order -- its FIFO is the arbiter,
            # so the W stream is never starved by X prefetches (a
            # front-loaded X stream lost ~6us to exactly that in profiling).
            x0t = wp.tile([128, 2, 128], bf16)
            nc.scalar.dma_start(out=x0t[:], in_=x0_d[:])
            nc.scalar.dma_start(out=wt[:, 0, 0:512], in_=w_d[0, :, 0:512])
            nc.scalar.dma_start(out=wt[:, 0, 512:], in_=w_d[0, :, 512:])
            xts = []
            for g in range(npair):
                xtn = xp.tile([128, 2, KCH, 128], bf16, tag="xt")
                nc.sync.dma_start(out=xtn[:], in_=xt_d[g])
                xts.append(xtn)
                if g == 0:
                    for k in range(1, KCH):
                        nc.sync.dma_start(out=wt[:, k, :], in_=w_d[k])

            for g in range(npair):
                xt = xts[g]
                tbs = [t for t in (2 * g, 2 * g + 1) if t < ntb]
                pss = [psp.tile([128, OUT_F], f32, name="ps", tag="ps")
                       for _ in tbs]
                for k in range(KCH):
                    for j, ps in enumerate(pss):
                        stat = x0t[:, j, :] if (g == 0 and k == 0) \
                            else xt[:, j, k, :]
                        for nh in range(2):
                            nc.tensor.matmul(
                                ps[:, nh * 512:(nh + 1) * 512],
                                stat,
                                wt[:, k, nh * 512:(nh + 1) * 512],
                                start=(k == 0),
                                stop=(k == KCH - 1),
                            )
                for j, ps in enumerate(pss):
                    tb = tbs[j]
                    ot = op.tile([128, OUT_F], f32)
                    rows = slice(tb * 128, (tb + 1) * 128)
                    if tb == ntb - 1:
                        # last block: quarter-grained copies ping-ponged
                        # across both PSUM-capable engines and both rings so
                        # the post-loop drain is ~128KB deep, not 512KB
                        nc.scalar.copy(ot[:, 0:256], ps[:, 0:256])
                        nc.sync.dma_start(out=out_d[rows, 0:256],
                                          in_=ot[:, 0:256])
                        nc.vector.tensor_copy(ot[:, 512:768], ps[:, 512:768])
                        nc.scalar.dma_start(out=out_d[rows, 512:768],
                                            in_=ot[:, 512:768])
                        nc.scalar.copy(ot[:, 256:512], ps[:, 256:512])
                        nc.sync.dma_start(out=out_d[rows, 256:512],
                                          in_=ot[:, 256:512])
                        nc.vector.tensor_copy(ot[:, 768:], ps[:, 768:])
                        nc.scalar.dma_start(out=out_d[rows, 768:],
                                            in_=ot[:, 768:])
                    else:
                        nc.scalar.copy(ot[:, 0:512], ps[:, 0:512])
                        nc.sync.dma_start(out=out_d[rows, 0:512],
                                          in_=ot[:, 0:512])
                        nc.vector.tensor_copy(ot[:, 512:], ps[:, 512:])
                        nc.scalar.dma_start(out=out_d[rows, 512:],
                                            in_=ot[:, 512:])
    nc.compile()
    return nc


def _pack_core(x_slice: np.ndarray, w_g: np.ndarray, ntb: int):
    npair = (ntb + 1) // 2
    n = x_slice.shape[0]
    xp = np.zeros((npair * 256, IN_F), dtype=np.float32)
    xp[:n] = x_slice
    # [pair, j, tok, k, p] -> [pair, p, j, k, tok]
    xt = np.ascontiguousarray(
        xp.reshape(npair, 2, 128, KCH, 128).transpose(0, 4, 1, 3, 2)
        .astype(BF16)
    )
    x0 = np.ascontiguousarray(xt[0, :, :, 0, :])
    wt = np.ascontiguousarray(w_g.reshape(KCH, 128, OUT_F).astype(BF16))
    return xt, x0, wt


def kernel(hidden_states: np.ndarray, weight: np.ndarray, offsets: np.ndarray,
           _trace: bool = False):
    hs = np.ascontiguousarray(hidden_states, dtype=np.float32)
    w = np.ascontiguousarray(weight, dtype=np.float32)
    off = np.asarray(offsets).astype(np.int64)

    ends = np.clip(off, 0, TOKENS)
    starts = np.concatenate(([0], ends[:-1]))
    starts = np.minimum(starts, ends)
    ns = ends - starts

    ntb = max(1, int(-(-ns.max() // 128)))
    nc = build(ntb)

    in_maps = []
    for g in range(GROUPS):
        xt, x0, wt = _pack_core(hs[starts[g]:ends[g]], w[g], ntb)
        in_maps.append({"xt": xt, "x0": x0, "w": wt})

    res = run_bass_kernel_spmd(nc, in_maps, list(range(GROUPS)), trace=_trace)

    out = np.zeros((TOKENS, OUT_F), dtype=np.float32)
    for g in range(GROUPS):
        if ns[g] > 0:
            out[starts[g]:ends[g]] = res.results[g]["out"][:ns[g]]
    if _trace:
        return out, res
    return out


# revision 12
# speedup vs baseline: 1.0009x; 1.0009x over previous
"""Grouped (MoE-style) linear on 8 trn2 NeuronCores.

out[t] = hidden_states[t] @ weight[g(t)], where token t belongs to group g iff
offsets[g-1] <= t < offsets[g] (searchsorted right semantics; tokens at or past
offsets[-1] get zero output).

Strategy: expert-parallel. Core g owns weight[g] and the contiguous token run
of group g. Routing is done host-side (offsets are host data); each core runs
an identical Bass program: [ntb*128, 1024] x [1024, 1024] matmul in bf16
(inputs cast host-side; PSUM accumulation stays fp32, measured ~1.5e-3 relmax
for this distribution -- well inside the 2e-2 gate).

Performance structure (per core, ntb=16):
  - PE floor is 256 N=512 matmuls = 131072 cycles ~= 54.6us at 2.4 GHz; the
    kernel is tensor-bound, so everything else is pipelined around the MM
    stream.
  - k-outer loop over PAIRS of 128-token blocks: each k-step is 4 matmuls
    (2 blocks x 2 out-halves) accumulating into 4 PSUM banks, so compute
    starts as soon as the first 128-row weight chunk lands instead of after
    the full 2MB weight load.
  - X streams on the sync HWDGE ring as one 512KB DMA per block-pair (4KB
    per-partition runs); W streams on the scalar ring as per-k 256KB chunks,
    issued in exactly the order the k-loop consumes them.
  - Outputs drain progressively: per block, ACT copies PSUM[0:512] -> SBUF
    and DMAs it on the sync ring while DVE copies PSUM[512:1024] for the
    scalar ring, so the post-loop tail is one 256KB DMA, not 8MB.
"""
import numpy as np
import ml_dtypes

import concourse.bass as bass
import concourse.tile as tile
from concourse import bacc, mybir
from concourse.bass_utils import run_bass_kernel_spmd

GROUPS = 8
TOKENS = 16384
IN_F = 1024
OUT_F = 1024
KCH = IN_F // 128  # contraction chunks
BF16 = ml_dtypes.bfloat16


def build(ntb: int) -> bass.Bass:
    """One core's program: ntb 128-token blocks through a 1024x1024 expert."""
    f32 = mybir.dt.float32
    bf16 = mybir.dt.bfloat16
    npair = (ntb + 1) // 2
    nc = bacc.Bacc()
    # xt[pair, p, j, k, tok] = X[(2*pair+j)*128 + tok, k*128 + p]
    xt_d = nc.dram_tensor("xt", [npair, 128, 2, KCH, 128], bf16,
                          kind="ExternalInput")
    # x0 duplicates pair 0's k=0 chunk contiguously (512B/partition runs;
    # slicing it out of xt would cost 256B-run descriptors on the hot path)
    x0_d = nc.dram_tensor("x0", [128, 2, 128], bf16, kind="ExternalInput")
    # w[k, p, n] = W[k*128 + p, n]
    w_d = nc.dram_tensor("w", [KCH, 128, OUT_F], bf16, kind="ExternalInput")
    out_d = nc.dram_tensor("out", [ntb * 128, OUT_F], f32,
                           kind="ExternalOutput")

    with tile.TileContext(nc) as tc:
        with (
            tc.tile_pool(name="wp", bufs=1) as wp,
            tc.tile_pool(name="xp", bufs=min(npair, 20)) as xp,
            tc.tile_pool(name="op", bufs=4) as op,
            tc.tile_pool(name="ps", bufs=4, space="PSUM") as psp,
        ):
            wt = wp.tile([128, KCH, OUT_F], bf16)
            # ALL inputs share the sync HWDGE ring, issued in exactly the
            # order the k-outer loop consumes them -- the ring's FIFO is the
            # arbiter, so the W stream is never starved by X prefetches
            # (a two-ring split loses ~50% of SDMA bandwidth to whichever
            # stream is ahead, measured twice).  Every DMA's semaphore fires
            # ~2us after its last byte (HBM receipt round-trip), so the
            # ordering below keeps each k-step's data a full step ahead.
            x0t = wp.tile([128, 2, 128], bf16)
            nc.sync.dma_start(out=x0t[:], in_=x0_d[:])
            nc.sync.dma_start(out=wt[:, 0, 0:512], in_=w_d[0, :, 0:512])
            nc.sync.dma_start(out=wt[:, 0, 512:], in_=w_d[0, :, 512:])
            xts = []
            xt0 = xp.tile([128, 2, KCH, 128], bf16, tag="xt")
            nc.sync.dma_start(out=xt0[:, 0], in_=xt_d[0, :, 0])
            nc.sync.dma_start(out=wt[:, 1, :], in_=w_d[1])
            nc.sync.dma_start(out=xt0[:, 1], in_=xt_d[0, :, 1])
            xts.append(xt0)
            for k in range(2, KCH):
                nc.sync.dma_start(out=wt[:, k, :], in_=w_d[k])
            for g in range(1, npair):
                xtn = xp.tile([128, 2, KCH, 128], bf16, tag="xt")
                nc.sync.dma_start(out=xtn[:], in_=xt_d[g])
                xts.append(xtn)

            for g in range(npair):
                xt = xts[g]
                tbs = [t for t in (2 * g, 2 * g + 1) if t < ntb]
                pss = [psp.tile([128, OUT_F], f32, name="ps", tag="ps")
                       for _ in tbs]
                for k in range(KCH):
                    for j, ps in enumerate(pss):
                        stat = x0t[:, j, :] if (g == 0 and k == 0) \
                            else xt[:, j, k, :]
                        for nh in range(2):
                            nc.tensor.matmul(
                                ps[:, nh * 512:(nh + 1) * 512],
                                stat,
                                wt[:, k, nh * 512:(nh + 1) * 512],
                                start=(k == 0),
                                stop=(k == KCH - 1),
                            )
                for j, ps in enumerate(pss):
                    tb = tbs[j]
                    ot = op.tile([128, OUT_F], f32)
                    rows = slice(tb * 128, (tb + 1) * 128)
                    if tb == ntb - 1:
                        # last block: quarter-grained copies ping-ponged
                        # across both PSUM-capable engines and both rings so
                        # the post-loop drain is ~128KB deep, not 512KB
                        nc.scalar.copy(ot[:, 0:256], ps[:, 0:256])
                        nc.sync.dma_start(out=out_d[rows, 0:256],
                                          in_=ot[:, 0:256])
                        nc.vector.tensor_copy(ot[:, 512:768], ps[:, 512:768])
                        nc.scalar.dma_start(out=out_d[rows, 512:768],
                                            in_=ot[:, 512:768])
                        nc.scalar.copy(ot[:, 256:512], ps[:, 256:512])
                        nc.sync.dma_start(out=out_d[rows, 256:512],
                                          in_=ot[:, 256:512])
                        nc.vector.tensor_copy(ot[:, 768:], ps[:, 768:])
                        nc.scalar.dma_start(out=out_d[rows, 768:],
                                            in_=ot[:, 768:])
                    else:
                        nc.scalar.copy(ot[:, 0:512], ps[:, 0:512])
                        nc.sync.dma_start(out=out_d[rows, 0:512],
                                          in_=ot[:, 0:512])
                        nc.vector.tensor_copy(ot[:, 512:], ps[:, 512:])
                        nc.scalar.dma_start(out=out_d[rows, 512:],
                                            in_=ot[:, 512:])
    nc.compile()
    return nc


def _pack_core(x_slice: np.ndarray, w_g: np.ndarray, ntb: int):
    npair = (ntb + 1) // 2
    n = x_slice.shape[0]
    xp = np.zeros((npair * 256, IN_F), dtype=np.float32)
    xp[:n] = x_slice
    # [pair, j, tok, k, p] -> [pair, p, j, k, tok]
    xt = np.ascontiguousarray(
        xp.reshape(npair, 2, 128, KCH, 128).transpose(0, 4, 1, 3, 2)
        .astype(BF16)
    )
    x0 = np.ascontiguousarray(xt[0, :, :, 0, :])
    wt = np.ascontiguousarray(w_g.reshape(KCH, 128, OUT_F).astype(BF16))
    return xt, x0, wt


def kernel(hidden_states: np.ndarray, weight: np.ndarray, offsets: np.ndarray,
           _trace: bool = False):
    hs = np.ascontiguousarray(hidden_states, dtype=np.float32)
    w = np.ascontiguousarray(weight, dtype=np.float32)
    off = np.asarray(offsets).astype(np.int64)

    ends = np.clip(off, 0, TOKENS)
    starts = np.concatenate(([0], ends[:-1]))
    starts = np.minimum(starts, ends)
    ns = ends - starts

    ntb = max(1, int(-(-ns.max() // 128)))
    nc = build(ntb)

    in_maps = []
    for g in range(GROUPS):
        xt, x0, wt = _pack_core(hs[starts[g]:ends[g]], w[g], ntb)
        in_maps.append({"xt": xt, "x0": x0, "w": wt})

    res = run_bass_kernel_spmd(nc, in_maps, list(range(GROUPS)), trace=_trace)

    out = np.zeros((TOKENS, OUT_F), dtype=np.float32)
    for g in range(GROUPS):
        if ns[g] > 0:
            out[starts[g]:ends[g]] = res.results[g]["out"][:ns[g]]
    if _trace:
        return out, res
    return out
